# revision 8
# baseline (speedup 1.0000x reference)
"""Multi-head self-attention (causal) Trainium2 Bass kernel, 8-core SPMD.

Sharding: 8 cores = 2 batches x 4 head-groups (3 heads each).
Each core computes, for its (batch, head-group):
  - Q^T, K^T, V projections from a host-pretransposed x^T (bf16)
  - causal attention with scores kept transposed (S^T[k,q]) so no on-device
    transposes are needed; softmax denominator comes free via a ones-column
    appended to V
  - its 3 heads' slice of the output projection (partial sum over d)
Host gathers: out[b] = sum of 4 group partials + (b_proj + b_v @ W_proj).
b_k is dropped (softmax row-shift invariance), b_v folded into host bias.
"""

import numpy as np
import ml_dtypes

S = 2048          # sequence length
D = 768           # model dim
HD = 64           # head dim
HPC = 3           # heads per core
NCORES = 8
P = 128           # partitions
CT = D // P       # 6 contraction tiles over model dim
KT = S // P       # 16 key tiles
QC = 512          # query chunk (PSUM bank width in fp32)
NQC = S // QC     # 4 query chunks

_BF = ml_dtypes.bfloat16

_cache = {}


def _build_nc():
    import concourse.bass as bass
    import concourse.mybir as mybir
    import concourse.tile as tile
    from concourse import bacc
    from contextlib import ExitStack

    bf = mybir.dt.bfloat16
    f32 = mybir.dt.float32

    nc = bacc.Bacc()
    xT = nc.declare_dram_parameter("xT", [D, S], bf, isOutput=False)
    # 4 lhsT slots per c-tile: 0=[Wk0|Wk1] 1=[Wq0|Wq1] 2=[Wk2|0] 3=[Wq2|0]
    w_qk = nc.declare_dram_parameter("w_qk", [D, 4, P], bf, isOutput=False)
    w_v = nc.declare_dram_parameter("w_v", [D, HPC * HD], bf, isOutput=False)
    bq = nc.declare_dram_parameter("bq", [P, 2], f32, isOutput=False)
    w_p = nc.declare_dram_parameter("w_p", [HD, HPC, D], bf, isOutput=False)
    mask = nc.declare_dram_parameter("mask", [P, P], bf, isOutput=False)
    out_p = nc.declare_dram_parameter("out_p", [S, D], f32, isOutput=True)

    Exp = mybir.ActivationFunctionType.Exp

    with tile.TileContext(nc) as tc, ExitStack() as ctx:
        singles = ctx.enter_context(tc.tile_pool(name="singles", bufs=1))
        pmm = ctx.enter_context(tc.tile_pool(name="pmm", bufs=2, space="PSUM"))
        ps_pool = ctx.enter_context(tc.tile_pool(name="ps", bufs=2, space="PSUM"))
        po_pool = ctx.enter_context(tc.tile_pool(name="po", bufs=2, space="PSUM"))
        pp_pool = ctx.enter_context(tc.tile_pool(name="pp", bufs=2, space="PSUM"))
        pt_pool = ctx.enter_context(tc.tile_pool(name="pt", bufs=3))
        norm_pool = ctx.enter_context(tc.tile_pool(name="norm", bufs=2))
        outs_pool = ctx.enter_context(tc.tile_pool(name="outs", bufs=2))

        # ---- persistent SBUF ----
        xT_s = singles.tile([P, CT, S], bf)
        nc.sync.dma_start(out=xT_s, in_=xT.rearrange("(t p) q -> p t q", p=P))
        wqk_s = singles.tile([P, CT, 4, P], bf)
        nc.sync.dma_start(out=wqk_s, in_=w_qk.rearrange("(t p) s m -> p t s m", p=P))
        wv_s = singles.tile([P, CT, HPC * HD], bf)
        nc.sync.dma_start(out=wv_s, in_=w_v.rearrange("(t p) m -> p t m", p=P))
        bq_s = singles.tile([P, 2], f32)
        nc.sync.dma_start(out=bq_s, in_=bq[:])
        wp_s = singles.tile([HD, HPC, D], bf)
        nc.sync.dma_start(out=wp_s, in_=w_p[:])
        mask_s = singles.tile([P, P], bf)
        nc.sync.dma_start(out=mask_s, in_=mask[:])

        # Q^T/K^T: slot 0 holds head0 (parts 0:64) + head1 (parts 64:128),
        # slot 1 holds head2 (parts 0:64).
        qt_s = singles.tile([P, 2, S], bf)
        kt_s = singles.tile([P, 2, S], bf)
        # V with a ones column appended per head (softmax denominator trick)
        v_s = singles.tile([P, KT, HPC, HD + 1], bf)
        nc.vector.memset(v_s[:, :, :, HD:HD + 1], 1.0)
        attn_s = singles.tile([HD, HPC, S], bf)

        def head_qk(h):
            """(qt/kt slot, partition slice) for head h."""
            if h == 0:
                return 0, slice(0, HD)
            if h == 1:
                return 0, slice(HD, P)
            return 1, slice(0, HD)

        for c in range(NQC):
            qs = c * QC
            qsl = slice(qs, qs + QC)
            # ---- QKV projections for this q/k chunk ----
            ps_kk = pmm.tile([P, QC], mybir.dt.float32, tag="mm")
            for ct in range(CT):
                nc.tensor.matmul(ps_kk, lhsT=wqk_s[:, ct, 0, :],
                                 rhs=xT_s[:, ct, qsl],
                                 start=(ct == 0), stop=(ct == CT - 1))
            nc.vector.tensor_copy(out=kt_s[:, 0, qsl], in_=ps_kk)

            ps_qq = pmm.tile([P, QC], mybir.dt.float32, tag="mm")
            for ct in range(CT):
                nc.tensor.matmul(ps_qq, lhsT=wqk_s[:, ct, 1, :],
                                 rhs=xT_s[:, ct, qsl],
                                 start=(ct == 0), stop=(ct == CT - 1))
            nc.scalar.activation(out=qt_s[:, 0, qsl], in_=ps_qq,
                                 func=mybir.ActivationFunctionType.Identity,
                                 bias=bq_s[:, 0:1], scale=1.0)

            ps_k2 = pmm.tile([P, QC], mybir.dt.float32, tag="mm")
            for ct in range(CT):
                nc.tensor.matmul(ps_k2[0:HD, :], lhsT=wqk_s[:, ct, 2, 0:HD],
                                 rhs=xT_s[:, ct, qsl],
                                 start=(ct == 0), stop=(ct == CT - 1))
            nc.vector.tensor_copy(out=kt_s[0:HD, 1, qsl], in_=ps_k2[0:HD, :])

            ps_q2 = pmm.tile([P, QC], mybir.dt.float32, tag="mm")
            for ct in range(CT):
                nc.tensor.matmul(ps_q2[0:HD, :], lhsT=wqk_s[:, ct, 3, 0:HD],
                                 rhs=xT_s[:, ct, qsl],
                                 start=(ct == 0), stop=(ct == CT - 1))
            nc.scalar.activation(out=qt_s[0:HD, 1, qsl], in_=ps_q2[0:HD, :],
                                 func=mybir.ActivationFunctionType.Identity,
                                 bias=bq_s[0:HD, 1:2], scale=1.0)

            # V for the 4 k-tiles of this chunk
            for kt in range(4 * c, 4 * c + 4):
                ps_v = pmm.tile([P, QC], mybir.dt.float32, tag="mm")
                for ct in range(CT):
                    nc.tensor.matmul(ps_v[:, 0:HPC * HD], lhsT=xT_s[:, ct, kt * P:(kt + 1) * P],
                                     rhs=wv_s[:, ct, :],
                                     start=(ct == 0), stop=(ct == CT - 1))
                for h in range(HPC):
                    nc.vector.tensor_copy(out=v_s[:, kt, h, 0:HD],
                                          in_=ps_v[:, h * HD:(h + 1) * HD])

            # ---- attention for q-chunk c ----
            for h in range(HPC):
                slot, psl = head_qk(h)
                po = po_pool.tile([P, QC], mybir.dt.float32, tag="po")
                nkt = 4 * c + 4  # k-tiles participating
                for kt in range(nkt):
                    off = max(0, kt * P - qs)
                    n = QC - off
                    pt = pt_pool.tile([P, QC], bf, tag="pt")
                    ps_s = ps_pool.tile([P, QC], mybir.dt.float32, tag="ss")
                    nc.tensor.matmul(ps_s[:, 0:n],
                                     lhsT=kt_s[psl, slot, kt * P:(kt + 1) * P],
                                     rhs=qt_s[psl, slot, qs + off:qs + QC],
                                     start=True, stop=True)
                    if off > 0:
                        nc.vector.memset(pt[:, 0:off], 0.0)
                    nc.scalar.activation(out=pt[:, off:QC], in_=ps_s[:, 0:n],
                                         func=Exp, scale=0.125)
                    if kt * P >= qs:  # diagonal tile: mask k>q
                        nc.vector.tensor_mul(out=pt[:, off:off + P],
                                             in0=pt[:, off:off + P], in1=mask_s)
                    nc.tensor.matmul(po[0:HD + 1, :], lhsT=v_s[:, kt, h, :],
                                     rhs=pt,
                                     start=(kt == 0), stop=(kt == nkt - 1))
                # normalize: recip of denominator row, broadcast, multiply
                recip = norm_pool.tile([HD + 1, QC], mybir.dt.float32, tag="recip")
                nc.vector.reciprocal(out=recip[HD:HD + 1, :], in_=po[HD:HD + 1, :])
                bcast = norm_pool.tile([HD, 1, QC], mybir.dt.float32, tag="bcast")
                rs = recip[HD:HD + 1, :]
                rep = bass.AP(tensor=rs.tensor, offset=rs.offset,
                              ap=[list(rs.ap[0]), [0, HD], list(rs.ap[1])])
                nc.sync.dma_start(out=bcast, in_=rep)
                nc.vector.tensor_mul(out=attn_s[:, h, qsl], in0=po[0:HD, :],
                                     in1=bcast[:, 0, :])

            # ---- output projection for the 4 q-tiles of this chunk ----
            for t in range(4 * c, 4 * c + 4):
                ob = outs_pool.tile([P, D], mybir.dt.float32, tag="ob")
                for e0, en in ((0, 512), (512, 256)):
                    pp = pp_pool.tile([P, 512], mybir.dt.float32, tag="pp")
                    for h in range(HPC):
                        nc.tensor.matmul(pp[:, 0:en],
                                         lhsT=attn_s[:, h, t * P:(t + 1) * P],
                                         rhs=wp_s[:, h, e0:e0 + en],
                                         start=(h == 0), stop=(h == HPC - 1))
                    nc.vector.tensor_copy(out=ob[:, e0:e0 + en], in_=pp[:, 0:en])
                nc.sync.dma_start(out=out_p[t * P:(t + 1) * P, :], in_=ob)

    nc.compile()
    return nc


def _prep_inputs(x, W_qkv, b_qkv, W_proj):
    """Build the 8 per-core input maps (all bf16 except biases)."""
    in_maps = []
    for cid in range(NCORES):
        b, g = divmod(cid, 4)
        hs = [g * HPC + i for i in range(HPC)]  # global head ids
        xT = np.ascontiguousarray(x[b].T).astype(_BF)

        def wslice(kind, h):  # kind 0=q 1=k 2=v
            return W_qkv[:, kind * D + h * HD:(kind * D + (h + 1) * HD)]

        w_qk = np.zeros((D, 4, P), dtype=np.float32)
        w_qk[:, 0, 0:HD] = wslice(1, hs[0])
        w_qk[:, 0, HD:P] = wslice(1, hs[1])
        w_qk[:, 1, 0:HD] = wslice(0, hs[0])
        w_qk[:, 1, HD:P] = wslice(0, hs[1])
        w_qk[:, 2, 0:HD] = wslice(1, hs[2])
        w_qk[:, 3, 0:HD] = wslice(0, hs[2])

        w_v = np.concatenate([wslice(2, h) for h in hs], axis=1)

        bq = np.zeros((P, 2), dtype=np.float32)
        bq[0:HD, 0] = b_qkv[hs[0] * HD:(hs[0] + 1) * HD]
        bq[HD:P, 0] = b_qkv[hs[1] * HD:(hs[1] + 1) * HD]
        bq[0:HD, 1] = b_qkv[hs[2] * HD:(hs[2] + 1) * HD]

        w_p = np.stack([W_proj[h * HD:(h + 1) * HD, :] for h in hs], axis=1)

        mask = np.triu(np.ones((P, P), dtype=np.float32))

        in_maps.append({
            "xT": xT,
            "w_qk": w_qk.astype(_BF),
            "w_v": w_v.astype(_BF),
            "bq": bq,
            "w_p": w_p.astype(_BF),
            "mask": mask.astype(_BF),
        })
    return in_maps


def _run(inputs, trace=False):
    from concourse.bass_utils import run_bass_kernel_spmd

    x = np.asarray(inputs["x"], dtype=np.float32)
    W_qkv = np.asarray(inputs["W_qkv"], dtype=np.float32)
    b_qkv = np.asarray(inputs["b_qkv"], dtype=np.float32)
    W_proj = np.asarray(inputs["W_proj"], dtype=np.float32)
    b_proj = np.asarray(inputs["b_proj"], dtype=np.float32)

    if "nc" not in _cache:
        _cache["nc"] = _build_nc()
    nc = _cache["nc"]

    in_maps = _prep_inputs(x, W_qkv, b_qkv, W_proj)
    res = run_bass_kernel_spmd(nc, in_maps, core_ids=list(range(NCORES)),
                               trace=trace)

    host_bias = b_proj + b_qkv[2 * D:3 * D] @ W_proj  # b_v folded through proj
    B = x.shape[0]
    out = np.zeros((B, S, D), dtype=np.float32)
    for cid in range(NCORES):
        b = cid // 4
        out[b] += res.results[cid]["out_p"]
    out += host_bias
    return out, res


def kernel(x, W_qkv, b_qkv, W_proj, b_proj):
    out, _ = _run({"x": x, "W_qkv": W_qkv, "b_qkv": b_qkv,
                   "W_proj": W_proj, "b_proj": b_proj})
    return out
